# revision 30
# baseline (speedup 1.0000x reference)
"""Multi-head attention (no-transpose head reshape) on 8 trn2 cores.

Problem: B=2, S=2048, D=1024, H=16, DH=64.
  query = q @ Wq + bq  (same for k, v)
  dq = query.reshape(B, H, S, DH)   # NO transpose: head h uses rows
                                    # [128h, 128(h+1)) of query, reinterpreted
                                    # as a [2048, 64] matrix.
  out[b,h] = softmax(dq_h @ dk_h.T / 8) @ dv_h

Sharding: 32 independent (b, h) tasks; core c gets b=c//4 and the 4 heads
4*(c%4)..4*(c%4)+3, i.e. rows 512*(c%4)..+512 of batch b. No collectives.

Per-core kernel (all matmuls bf16, fp32 PSUM accumulation):
  - inputs arrive host-transposed and bf16-cast (xT [1024, 512]) so the
    contraction dim (din) is on partitions. Wq/bq pre-scaled by 1/8 on host
    so the softmax exp needs no scale.
  - Q/K projections computed transposed: XqT[dout, m] = sum_din Wq[din,dout]
    * xT[din, m]; stored [128, 8, 512] where partition = (bq%2)*64 + d2,
    free = (bq//2, h*128 + a); the head-matrix slice is
    dq_h.T[d2, s2=16a+bq] = XqT[bq*64+d2, h*128+a].
  - V projection computed in normal orientation and evicted directly into
    AV-lhsT layout av_lhs[ak, h, bk, 0:64] = dv rows, with a ones column at
    index 64 so the AV matmul also produces softmax denominators.
  - scores computed transposed per (h, m-half H, k-group bk):
    S_T[k=16ak+bk, m=16aq+2po+H] via lhsT = XkT slice [64, 128] (XkTs, the
    partition-swapped copy, when bk%2 != H) and rhs = XqT slice [64, 4, 128];
    exp on ACT -> bf16 attnT; AV matmul accumulates outT_aug [65, 1024]
    over the 16 bk (natural-parity bk first).
  - finish: copy to bf16, PE-transpose [65,128] chunks -> [128, 65] (bf16);
    divide by col 64; one strided DMA per (h, H) into out[h, s2, d2].

Scheduling (v2): exp on ACT is the span bottleneck (~142us); everything is
ordered to keep ACT fed: input-DMA triggers spread across 5 engine queues,
K-swap DMAs on gpsimd (off scalar), only V(h0) projected before attention,
V(h1..3) matmuls drip-fed between attention bk-groups, each unit's finish
chain emitted after the next unit's first scores.
"""

import numpy as np

B, S, D, NH = 2, 2048, 1024, 16
DH = 64
NCORES = 8
HPC = NH * B // NCORES      # heads per core = 4
ROWS = HPC * (S // NH)      # projection rows per core = 512
PO = D // 128               # 8 din/dout tiles

_BUILT = {}


def _build_nc(reps=1, salt=0.0):
    if ("nc", reps, salt) in _BUILT:
        return _BUILT[("nc", reps, salt)]

    import concourse.bass as bass
    import concourse.bacc as bacc
    import concourse.tile as tile
    from concourse import mybir
    from concourse.masks import make_identity
    from contextlib import ExitStack

    f32 = mybir.dt.float32
    bf16 = mybir.dt.bfloat16
    Exp = mybir.ActivationFunctionType.Exp

    nc = bacc.Bacc("TRN2", target_bir_lowering=False, debug=False)

    qT = nc.dram_tensor("qT", [D, ROWS], bf16, kind="ExternalInput")
    kT = nc.dram_tensor("kT", [D, ROWS], bf16, kind="ExternalInput")
    vT = nc.dram_tensor("vT", [D, ROWS], bf16, kind="ExternalInput")
    Wq = nc.dram_tensor("Wq", [D, D], bf16, kind="ExternalInput")
    Wk = nc.dram_tensor("Wk", [D, D], bf16, kind="ExternalInput")
    Wv = nc.dram_tensor("Wv", [D, D], bf16, kind="ExternalInput")
    bq = nc.dram_tensor("bq", [128, PO], f32, kind="ExternalInput")
    bk = nc.dram_tensor("bk", [128, PO], f32, kind="ExternalInput")
    bv = nc.dram_tensor("bv", [1, D], f32, kind="ExternalInput")
    out = nc.dram_tensor("out", [HPC, S, DH], f32, kind="ExternalOutput")
    # out viewed so row s2 = 16a + r: index [h, a, r, d]
    out_w = out.ap().rearrange("h (a r) d -> h a r d", r=16)

    P = 128
    KB = S // P                # 16 k-groups (bk)
    MH = ROWS * 2              # m-half = 1024 columns

    with tile.TileContext(nc) as tc, ExitStack() as ctx:
        consts = ctx.enter_context(tc.tile_pool(name="consts", bufs=1))
        inputs_v = ctx.enter_context(tc.tile_pool(name="inputs_v", bufs=1))
        proj = ctx.enter_context(tc.tile_pool(name="proj", bufs=1))
        wrows = ctx.enter_context(tc.tile_pool(name="wrows", bufs=1))

        for _rep in range(reps):
            inputs_qk = tc.tile_pool(name="inputs_qk", bufs=1)
            iqk = inputs_qk.__enter__()

            xqT = iqk.tile([P, PO, ROWS], bf16, tag="xqT")
            xkT = iqk.tile([P, PO, ROWS], bf16, tag="xkT")
            xvT = inputs_v.tile([P, PO, ROWS], bf16, tag="xvT")
            XqT = proj.tile([P, PO, ROWS], bf16, tag="XqT")
            XkT = proj.tile([P, PO, ROWS], bf16, tag="XkT")
            XkTs = proj.tile([P, PO, ROWS], bf16, tag="XkTs")
            av_lhs = proj.tile([P, HPC, KB, DH + 1], bf16, tag="av_lhs")

            bq_sb = consts.tile([P, PO], f32, tag="bq")
            bk_sb = consts.tile([P, PO], f32, tag="bk")
            bv_sb = consts.tile([P, D], f32, tag="bv")
            ident = consts.tile([P, P], bf16, tag="ident")

            # --- input DMAs ---
            # Only sync/scalar (HWDGE) and gpsimd (SWDGE) can trigger DMAs.
            # Criticals first: Q path on sync, K path on scalar, V on gpsimd.
            xqsrc = qT.ap().rearrange("(t p) c -> p t c", p=P)
            xksrc = kT.ap().rearrange("(t p) c -> p t c", p=P)
            xvsrc = vT.ap().rearrange("(t p) c -> p t c", p=P)

            # Full-tensor weight tiles, DMA'd in dout-halves (1KB runs per
            # partition line) so the first projection chains start after
            # only half a weight matrix has landed.
            wq_w = wrows.tile([P, PO, D], bf16, tag="wq", name="wq_w")
            wk_w = wrows.tile([P, PO, D], bf16, tag="wk", name="wk_w")
            wv_w = wrows.tile([P, PO, D], bf16, tag="wv", name="wv_w")
            wqsrc = Wq.ap().rearrange("(t p) (half c) -> p t half c", p=P, half=2)
            wksrc = Wk.ap().rearrange("(t p) (half c) -> p t half c", p=P, half=2)
            wvsrc = Wv.ap().rearrange("(t p) (half c) -> p t half c", p=P, half=2)
            HD = D // 2
            bv_ap = bv.ap()
            bv_bcast = bass.AP(tensor=bv_ap.tensor, offset=bv_ap.offset,
                               ap=[[0, P], [1, D]])

            # Staged: critical path to the first exp is xq + Wq (Q proj),
            # then xk head-0 slice + Wk (K head-0 proj). The V path rides
            # gpsimd in parallel (needed a few bk into the span).
            nc.sync.dma_start(out=xqT[:, 0:4, :], in_=xqsrc[:, 0:4, :])
            nc.sync.dma_start(out=xqT[:, 4:8, :], in_=xqsrc[:, 4:8, :])
            nc.sync.dma_start(out=wq_w[:, :, 0:HD], in_=wqsrc[:, :, 0, :])
            nc.sync.dma_start(out=wq_w[:, :, HD:D], in_=wqsrc[:, :, 1, :])
            nc.sync.dma_start(out=xkT[:, :, 0:P], in_=xksrc[:, :, 0:P])
            nc.sync.dma_start(out=wk_w[:, :, 0:HD], in_=wksrc[:, :, 0, :])
            nc.sync.dma_start(out=wk_w[:, :, HD:D], in_=wksrc[:, :, 1, :])
            nc.sync.dma_start(out=bq_sb[:], in_=bq[:])
            nc.sync.dma_start(out=bk_sb[:], in_=bk[:])
            nc.sync.dma_start(out=xkT[:, :, P:ROWS], in_=xksrc[:, :, P:ROWS])
            # V path on gpsimd
            nc.gpsimd.dma_start(out=xvT[:, 0:4, :], in_=xvsrc[:, 0:4, :])
            nc.gpsimd.dma_start(out=xvT[:, 4:8, :], in_=xvsrc[:, 4:8, :])
            nc.gpsimd.dma_start(out=wv_w[:, :, 0:HD], in_=wvsrc[:, :, 0, :])
            nc.gpsimd.dma_start(out=wv_w[:, :, HD:D], in_=wvsrc[:, :, 1, :])
            nc.gpsimd.dma_start(out=bv_sb[:], in_=bv_bcast)

            make_identity(nc, ident[:])
            warm = consts.tile([1, 1], f32, tag="warm")
            nc.vector.memset(warm[:], salt)
            nc.scalar.activation(warm[:], warm[:], Exp, scale=1.0)
            nc.vector.memset(av_lhs[:, :, :, DH:DH + 1], 1.0)

            # ---------------- projections: Q full + K head-0 ----------------
            # Q projects all 512 rows (M=512 chains). K projects only head
            # 0's 128 m-columns here (M=128) so attention starts early; the
            # remaining K columns are drip-fed into the attention stream.
            with tc.tile_pool(name="pj_ps", bufs=2, space="PSUM") as pj_ps:
                for po in range(PO):
                    ps = pj_ps.tile([P, ROWS], f32, tag="pjps",
                                    name=f"qpq{po}")
                    for dint in range(PO):
                        nc.tensor.matmul(
                            ps[:],
                            wq_w[:, dint, po * P:(po + 1) * P],
                            xqT[:, dint, :],
                            start=(dint == 0), stop=(dint == PO - 1))
                    nc.vector.tensor_scalar_add(XqT[:, po, :], ps[:],
                                                bq_sb[:, po:po + 1])
                for po in range(PO):
                    ps = pj_ps.tile([P, ROWS], f32, tag="pjps",
                                    name=f"kpq{po}")
                    for dint in range(PO):
                        nc.tensor.matmul(
                            ps[:, 0:P],
                            wk_w[:, dint, po * P:(po + 1) * P],
                            xkT[:, dint, 0:P],
                            start=(dint == 0), stop=(dint == PO - 1))
                    nc.vector.tensor_scalar_add(XkT[:, po, 0:P], ps[:, 0:P],
                                                bk_sb[:, po:po + 1])
                # head-0 partition swap (one pair of strided DMAs)
                nc.scalar.dma_start(out=XkTs[0:64, :, 0:P],
                                    in_=XkT[64:128, :, 0:P])
                nc.scalar.dma_start(out=XkTs[64:128, :, 0:P],
                                    in_=XkT[0:64, :, 0:P])

            # ------------- attention (V-proj drip-fed) -------------
            with tc.tile_pool(name="sc_ps", bufs=2, space="PSUM") as sc_ps, \
                 tc.tile_pool(name="av_ps", bufs=1, space="PSUM") as av_ps, \
                 tc.tile_pool(name="pv_tr", bufs=2, space="PSUM") as pv_tr, \
                 tc.tile_pool(name="attn", bufs=24) as attn_pool, \
                 tc.tile_pool(name="fin", bufs=4) as fin_pool:

                # Two deferred-work queues drained between bk groups:
                #   av_queue: AV matmuls (exactly 1 per bk -> AV lags scores
                #             by a few bk, so the PE never stalls on exp)
                #   bg_queue: V-proj chunks, finish chains, out-DMAs
                av_queue = []
                bg_queue = []

                def drain_av():
                    if av_queue:
                        av_queue.pop(0)()

                def drain_bg(n=1):
                    n += (len(bg_queue) > 12)
                    for _ in range(n):
                        if bg_queue:
                            bg_queue.pop(0)()

                # V projection: fine-grained items (2 matmuls each, ~210ns)
                # so a drain slot never displaces a score matmul by much.
                def v_proj_items(h):
                    items = []
                    state = {}
                    for dhalf in range(2):
                        for qh in range(2):
                            qd = dhalf * 2 + qh
                            for dp in range(PO // 2):
                                def mm(h=h, dhalf=dhalf, qh=qh, qd=qd, dp=dp):
                                    if dhalf not in state:
                                        state[dhalf] = pv_tr.tile(
                                            [P, ROWS], f32, tag="pvtr",
                                            name=f"vps{h}_{dhalf}")
                                    ps = state[dhalf]
                                    for dint in (2 * dp, 2 * dp + 1):
                                        nc.tensor.matmul(
                                            ps[:, qh * (ROWS // 2):(qh + 1) * (ROWS // 2)],
                                            xvT[:, dint, h * P:(h + 1) * P],
                                            wv_w[:, dint, qd * (D // 4):(qd + 1) * (D // 4)],
                                            start=(dint == 0),
                                            stop=(dint == PO - 1))
                                items.append(mm)

                        def evict(h=h, dhalf=dhalf):
                            ps = state.pop(dhalf)
                            nc.vector.tensor_add(
                                av_lhs[:, h, dhalf * 8:(dhalf + 1) * 8, 0:DH],
                                ps[:],
                                bv_sb[:, dhalf * ROWS:(dhalf + 1) * ROWS])
                        items.append(evict)
                    return items

                # K-rest projection drip items, per head block (hb=1 alone
                # first — its deadline is unit h1 — then h2+h3 together).
                def k_rest_items(m0, m1):
                    items = []
                    state = {}
                    mw = m1 - m0
                    for po in range(PO):
                        for dp in range(2):
                            def mm(po=po, dp=dp):
                                if po not in state:
                                    state[po] = pv_tr.tile(
                                        [P, ROWS], f32, tag="pvtr",
                                        name=f"krest{m0}_{po}")
                                ps = state[po]
                                for dint in range(4 * dp, 4 * dp + 4):
                                    nc.tensor.matmul(
                                        ps[:, 0:mw],
                                        wk_w[:, dint, po * P:(po + 1) * P],
                                        xkT[:, dint, m0:m1],
                                        start=(dint == 0),
                                        stop=(dint == PO - 1))
                            items.append(mm)

                        def evict_swap(po=po):
                            ps = state.pop(po)
                            nc.vector.tensor_scalar_add(
                                XkT[:, po, m0:m1], ps[:, 0:mw],
                                bk_sb[:, po:po + 1])
                            nc.gpsimd.dma_start(out=XkTs[0:64, po, m0:m1],
                                                in_=XkT[64:128, po, m0:m1])
                            nc.gpsimd.dma_start(out=XkTs[64:128, po, m0:m1],
                                                in_=XkT[0:64, po, m0:m1])
                        items.append(evict_swap)
                    return items

                # V(h0) fully before attention.
                for item in v_proj_items(0):
                    item()

                def unit(h, Hh, last=False, mid=None):
                    out_ps = av_ps.tile([DH + 1, MH], f32, tag="avps")
                    bk_order = ([b_ for b_ in range(KB) if b_ % 2 == Hh] +
                                [b_ for b_ in range(KB) if b_ % 2 != Hh])
                    for bki, bkk in enumerate(bk_order):
                        if bki == 6 and mid:
                            bg_queue.extend(mid)
                        ksrc = XkT if (bkk % 2) == Hh else XkTs
                        lhsT = ksrc[Hh * 64:Hh * 64 + 64, bkk // 2,
                                    h * P:(h + 1) * P]
                        s_ps = sc_ps.tile([P, MH], f32, tag="scps")
                        for j2 in range(2):
                            rhs = XqT[Hh * 64:Hh * 64 + 64,
                                      4 * j2:4 * j2 + 4, h * P:(h + 1) * P]
                            nc.tensor.matmul(
                                s_ps[:, j2 * ROWS:(j2 + 1) * ROWS],
                                lhsT, rhs, start=True, stop=True)
                        at = attn_pool.tile([P, MH], bf16, tag="at")
                        nc.scalar.activation(at[:], s_ps[:], Exp, scale=1.0)

                        def av(bki=bki, bkk=bkk, at=at, out_ps=out_ps, h=h):
                            for j2 in range(2):
                                nc.tensor.matmul(
                                    out_ps[:, j2 * ROWS:(j2 + 1) * ROWS],
                                    av_lhs[:, h, bkk, :],
                                    at[:, j2 * ROWS:(j2 + 1) * ROWS],
                                    start=(bki == 0), stop=(bki == KB - 1))
                        av_queue.append(av)
                        drain_av()
                        drain_bg(3 if last else 1)

                    # finish chain: deferred (drains during the next unit).
                    # copy_out has a hard deadline (av_ps bufs=1 WAR with the
                    # next unit's first AV) so unit() queues it itself; the
                    # caller queues fin chunks after any V-proj items.
                    def copy_out(out_ps=out_ps, h=h, Hh=Hh):
                        oT = fin_pool.tile([DH + 1, MH], bf16, tag="oT", bufs=2)
                        nc.vector.tensor_copy(oT[:], out_ps[:])
                        finish_state[(h, Hh)] = oT

                    def fin_chunk(j, h=h, Hh=Hh):
                        def run():
                            oT = finish_state[(h, Hh)]
                            stage = finish_state.get((h, Hh, "stage"))
                            if stage is None:
                                stage = fin_pool.tile([P, MH // P, DH], f32,
                                                      tag="stage", bufs=2,
                                                      name=f"stage{h}_{Hh}")
                                finish_state[(h, Hh, "stage")] = stage
                            tp = pv_tr.tile([P, DH + 1], bf16, tag="pvtr")
                            nc.tensor.transpose(tp[:], oT[:, j * P:(j + 1) * P],
                                                ident[0:DH + 1, 0:DH + 1])
                            rcp = fin_pool.tile([P, 1], f32, tag="rcp")
                            nc.vector.reciprocal(rcp[:], tp[:, DH:DH + 1])
                            nc.vector.tensor_scalar_mul(stage[:, j, :],
                                                        tp[:, 0:DH], rcp[:])
                        return run

                    def dma_out(h=h, Hh=Hh):
                        stage = finish_state.pop((h, Hh, "stage"))
                        finish_state.pop((h, Hh))
                        nc.sync.dma_start(out=out_w[h, :, Hh::2, :], in_=stage[:])

                    # queue-jump: copy_out must drain at the very next bk
                    # (av_ps bufs=1 WAR with the next unit's first AV)
                    bg_queue.insert(0, copy_out)
                    return [fin_chunk(j) for j in range(MH // P)] + [dma_out]

                finish_state = {}
                pending = []
                # drip schedule balanced so no unit gets more than ~32 items:
                #   (h0,0): K-h1   (h0,1): V-h1   (h1,0): K-h2h3
                #   (h1,1): V-h2   (h2,1): V-h3
                for h in range(HPC):
                    for Hh in range(2):
                        mid = None
                        if h == 0 and Hh == 0:
                            # delayed to bk6: xk-rest DMA must land first
                            mid = k_rest_items(P, 2 * P)
                        elif h == 1 and Hh == 0:
                            bg_queue.extend(k_rest_items(2 * P, ROWS))
                        elif Hh == 1 and h + 1 < HPC:
                            bg_queue.extend(v_proj_items(h + 1))
                        bg_queue.extend(pending)
                        pending = unit(h, Hh, last=(h == HPC - 1), mid=mid)
                # tail: drain everything that's left
                while av_queue:
                    drain_av()
                bg_queue.extend(pending)
                while bg_queue:
                    bg_queue.pop(0)()

            inputs_qk.__exit__(None, None, None)

    nc.compile()
    _dedupe_ldweights(nc)
    _BUILT[("nc", reps, salt)] = nc
    return nc


def _dedupe_ldweights(nc):
    """Remove InstLdweights that reload the stationary already resident in
    the PE array (consecutive matmuls sharing lhsT). Conservative: only
    sync-free duplicates; tracking resets at transposes (which clobber the
    array), drains, branches and any synced reload."""
    def key(a):
        return (str(a.memref), a.offset, str(a.ap), str(a.dtype))

    for f in nc.m.functions:
        for b in f.blocks:
            last = None
            keep = []
            for i in b.instructions:
                tn = type(i).__name__
                if tn == "InstLdweights":
                    k = key(i.ins[0])
                    si = i.sync_info
                    clean = (si is None) or (not si.on_wait and not si.on_update)
                    if last == k and clean:
                        continue
                    last = k
                elif tn == "InstMatmult":
                    if i.is_transpose:
                        last = None
                elif tn in ("InstDrain", "InstUnconditionalBranch", "InstCall"):
                    last = None
                keep.append(i)
            b.instructions[:] = keep


def _make_in_maps(q, k, v, Wq, bq, Wk, bk, Wv, bv):
    import ml_dtypes
    bfl = ml_dtypes.bfloat16

    q = np.asarray(q, dtype=np.float32)
    k = np.asarray(k, dtype=np.float32)
    v = np.asarray(v, dtype=np.float32)
    # fold the 1/sqrt(DH)=1/8 softmax scale into Wq/bq (exact: power of 2)
    Wq_b = np.ascontiguousarray((np.asarray(Wq, np.float32) * 0.125).astype(bfl))
    Wk_b = np.ascontiguousarray(np.asarray(Wk, np.float32).astype(bfl))
    Wv_b = np.ascontiguousarray(np.asarray(Wv, np.float32).astype(bfl))
    bq_t = np.ascontiguousarray(
        (np.asarray(bq, np.float32) * 0.125).reshape(PO, 128).T)
    bk_t = np.ascontiguousarray(np.asarray(bk, np.float32).reshape(PO, 128).T)
    bv_t = np.ascontiguousarray(np.asarray(bv, np.float32).reshape(1, D))

    in_maps = []
    for c in range(NCORES):
        b = c // (NCORES // B)
        r0 = (c % (NCORES // B)) * ROWS
        in_maps.append({
            "qT": np.ascontiguousarray(q[b, r0:r0 + ROWS, :].T.astype(bfl)),
            "kT": np.ascontiguousarray(k[b, r0:r0 + ROWS, :].T.astype(bfl)),
            "vT": np.ascontiguousarray(v[b, r0:r0 + ROWS, :].T.astype(bfl)),
            "Wq": Wq_b, "Wk": Wk_b, "Wv": Wv_b,
            "bq": bq_t, "bk": bk_t, "bv": bv_t,
        })
    return in_maps


def kernel(q, k, v, Wq, bq, Wk, bk, Wv, bv):
    from concourse.bass_utils import run_bass_kernel_spmd

    nc = _build_nc()
    in_maps = _make_in_maps(q, k, v, Wq, bq, Wk, bk, Wv, bv)
    res = run_bass_kernel_spmd(nc, in_maps, core_ids=list(range(NCORES)))

    outp = np.empty((B, NH, S, DH), dtype=np.float32)
    for c in range(NCORES):
        b = c // (NCORES // B)
        h0 = (c % (NCORES // B)) * HPC
        outp[b, h0:h0 + HPC] = res.results[c]["out"]
    return outp


# revision 31
# speedup vs baseline: 1.1873x; 1.1873x over previous
"""Multi-head attention (no-transpose head reshape) on 8 trn2 cores.

Problem: B=2, S=2048, D=1024, H=16, DH=64.
  query = q @ Wq + bq  (same for k, v)
  dq = query.reshape(B, H, S, DH)   # NO transpose: head h uses rows
                                    # [128h, 128(h+1)) of query, reinterpreted
                                    # as a [2048, 64] matrix.
  out[b,h] = softmax(dq_h @ dk_h.T / 8) @ dv_h

Sharding: 32 independent (b, h) tasks; core c gets b=c//4 and the 4 heads
4*(c%4)..4*(c%4)+3, i.e. rows 512*(c%4)..+512 of batch b. No collectives.

Per-core kernel (all matmuls bf16, fp32 PSUM accumulation):
  - inputs arrive host-transposed and bf16-cast (xT [1024, 512]) so the
    contraction dim (din) is on partitions. Wq/bq pre-scaled by 1/8 on host
    so the softmax exp needs no scale.
  - Q/K projections computed transposed: XqT[dout, m] = sum_din Wq[din,dout]
    * xT[din, m]; stored [128, 8, 512] where partition = (bq%2)*64 + d2,
    free = (bq//2, h*128 + a); the head-matrix slice is
    dq_h.T[d2, s2=16a+bq] = XqT[bq*64+d2, h*128+a].
  - V projection computed in normal orientation and evicted directly into
    AV-lhsT layout av_lhs[ak, h, bk, 0:64] = dv rows, with a ones column at
    index 64 so the AV matmul also produces softmax denominators.
  - scores computed transposed per (h, m-half H, k-group bk):
    S_T[k=16ak+bk, m=16aq+2po+H] via lhsT = XkT slice [64, 128] (XkTs, the
    partition-swapped copy, when bk%2 != H) and rhs = XqT slice [64, 4, 128];
    exp on ACT -> bf16 attnT; AV matmul accumulates outT_aug [65, 1024]
    over the 16 bk (natural-parity bk first).
  - finish: copy to bf16, PE-transpose [65,128] chunks -> [128, 65] (bf16);
    divide by col 64; one strided DMA per (h, H) into out[h, s2, d2].

Scheduling (v2): exp on ACT is the span bottleneck (~142us); everything is
ordered to keep ACT fed: input-DMA triggers spread across 5 engine queues,
K-swap DMAs on gpsimd (off scalar), only V(h0) projected before attention,
V(h1..3) matmuls drip-fed between attention bk-groups, each unit's finish
chain emitted after the next unit's first scores.
"""

import numpy as np

B, S, D, NH = 2, 2048, 1024, 16
DH = 64
NCORES = 8
HPC = NH * B // NCORES      # heads per core = 4
ROWS = HPC * (S // NH)      # projection rows per core = 512
PO = D // 128               # 8 din/dout tiles

_BUILT = {}


def _build_nc(reps=1, salt=0.0):
    if ("nc", reps, salt) in _BUILT:
        return _BUILT[("nc", reps, salt)]

    import concourse.bass as bass
    import concourse.bacc as bacc
    import concourse.tile as tile
    from concourse import mybir
    from concourse.masks import make_identity
    from contextlib import ExitStack

    f32 = mybir.dt.float32
    bf16 = mybir.dt.bfloat16
    Exp = mybir.ActivationFunctionType.Exp

    nc = bacc.Bacc("TRN2", target_bir_lowering=False, debug=False)

    qT = nc.dram_tensor("qT", [D, ROWS], bf16, kind="ExternalInput")
    kT = nc.dram_tensor("kT", [D, ROWS], bf16, kind="ExternalInput")
    vT = nc.dram_tensor("vT", [D, ROWS], bf16, kind="ExternalInput")
    Wq = nc.dram_tensor("Wq", [D, D], bf16, kind="ExternalInput")
    Wk = nc.dram_tensor("Wk", [D, D], bf16, kind="ExternalInput")
    Wv = nc.dram_tensor("Wv", [D, D], bf16, kind="ExternalInput")
    bq = nc.dram_tensor("bq", [128, PO], f32, kind="ExternalInput")
    bk = nc.dram_tensor("bk", [128, PO], f32, kind="ExternalInput")
    bv = nc.dram_tensor("bv", [1, D], f32, kind="ExternalInput")
    out = nc.dram_tensor("out", [HPC, S, DH], f32, kind="ExternalOutput")
    # out viewed so row s2 = 16a + r: index [h, a, r, d]
    out_w = out.ap().rearrange("h (a r) d -> h a r d", r=16)

    P = 128
    KB = S // P                # 16 k-groups (bk)
    MH = ROWS * 2              # m-half = 1024 columns

    with tile.TileContext(nc) as tc, ExitStack() as ctx:
        consts = ctx.enter_context(tc.tile_pool(name="consts", bufs=1))
        inputs_v = ctx.enter_context(tc.tile_pool(name="inputs_v", bufs=1))
        proj = ctx.enter_context(tc.tile_pool(name="proj", bufs=1))
        wrows = ctx.enter_context(tc.tile_pool(name="wrows", bufs=1))

        for _rep in range(reps):
            inputs_qk = tc.tile_pool(name="inputs_qk", bufs=1)
            iqk = inputs_qk.__enter__()

            xqT = iqk.tile([P, PO, ROWS], bf16, tag="xqT")
            xkT = iqk.tile([P, PO, ROWS], bf16, tag="xkT")
            xvT = inputs_v.tile([P, PO, ROWS], bf16, tag="xvT")
            XqT = proj.tile([P, PO, ROWS], bf16, tag="XqT")
            XkT = proj.tile([P, PO, ROWS], bf16, tag="XkT")
            XkTs = proj.tile([P, PO, ROWS], bf16, tag="XkTs")
            av_lhs = proj.tile([P, HPC, KB, DH + 1], bf16, tag="av_lhs")

            bq_sb = consts.tile([P, PO], f32, tag="bq")
            bk_sb = consts.tile([P, PO], f32, tag="bk")
            bv_sb = consts.tile([P, D], f32, tag="bv")
            ident = consts.tile([P, P], bf16, tag="ident")

            # --- input DMAs ---
            # Only sync/scalar (HWDGE) and gpsimd (SWDGE) can trigger DMAs.
            # Criticals first: Q path on sync, K path on scalar, V on gpsimd.
            xqsrc = qT.ap().rearrange("(t p) c -> p t c", p=P)
            xksrc = kT.ap().rearrange("(t p) c -> p t c", p=P)
            xvsrc = vT.ap().rearrange("(t p) c -> p t c", p=P)

            # Full-tensor weight tiles, DMA'd in dout-halves (1KB runs per
            # partition line) so the first projection chains start after
            # only half a weight matrix has landed.
            wq_w = wrows.tile([P, PO, D], bf16, tag="wq", name="wq_w")
            wk_w = wrows.tile([P, PO, D], bf16, tag="wk", name="wk_w")
            wv_w = wrows.tile([P, PO, D], bf16, tag="wv", name="wv_w")
            wqsrc = Wq.ap().rearrange("(t p) (half c) -> p t half c", p=P, half=2)
            wksrc = Wk.ap().rearrange("(t p) (half c) -> p t half c", p=P, half=2)
            wvsrc = Wv.ap().rearrange("(t p) (half c) -> p t half c", p=P, half=2)
            HD = D // 2
            bv_ap = bv.ap()
            bv_bcast = bass.AP(tensor=bv_ap.tensor, offset=bv_ap.offset,
                               ap=[[0, P], [1, D]])

            # Staged: critical path to the first exp is xq + Wq (Q proj),
            # then xk head-0 slice + Wk (K head-0 proj). The V path rides
            # gpsimd in parallel (needed a few bk into the span).
            nc.sync.dma_start(out=xqT[:, 0:4, :], in_=xqsrc[:, 0:4, :])
            nc.sync.dma_start(out=xqT[:, 4:8, :], in_=xqsrc[:, 4:8, :])
            nc.sync.dma_start(out=wq_w[:, :, 0:HD], in_=wqsrc[:, :, 0, :])
            nc.sync.dma_start(out=wq_w[:, :, HD:D], in_=wqsrc[:, :, 1, :])
            nc.sync.dma_start(out=bq_sb[:], in_=bq[:])
            nc.sync.dma_start(out=xkT[:, 0:4, :], in_=xksrc[:, 0:4, :])
            nc.sync.dma_start(out=xkT[:, 4:8, :], in_=xksrc[:, 4:8, :])
            nc.sync.dma_start(out=wk_w[:, :, 0:HD], in_=wksrc[:, :, 0, :])
            nc.sync.dma_start(out=wk_w[:, :, HD:D], in_=wksrc[:, :, 1, :])
            nc.sync.dma_start(out=bk_sb[:], in_=bk[:])
            # V path on gpsimd
            nc.gpsimd.dma_start(out=xvT[:, 0:4, :], in_=xvsrc[:, 0:4, :])
            nc.gpsimd.dma_start(out=xvT[:, 4:8, :], in_=xvsrc[:, 4:8, :])
            nc.gpsimd.dma_start(out=wv_w[:, :, 0:HD], in_=wvsrc[:, :, 0, :])
            nc.gpsimd.dma_start(out=wv_w[:, :, HD:D], in_=wvsrc[:, :, 1, :])
            nc.gpsimd.dma_start(out=bv_sb[:], in_=bv_bcast)

            make_identity(nc, ident[:])
            warm = consts.tile([1, 1], f32, tag="warm")
            nc.vector.memset(warm[:], salt)
            nc.scalar.activation(warm[:], warm[:], Exp, scale=1.0)
            nc.vector.memset(av_lhs[:, :, :, DH:DH + 1], 1.0)

            # ---------------- projections: Q full + K head-0 ----------------
            # Q projects all 512 rows (M=512 chains). K projects only head
            # 0's 128 m-columns here (M=128) so attention starts early; the
            # remaining K columns are drip-fed into the attention stream.
            with tc.tile_pool(name="pj_ps", bufs=2, space="PSUM") as pj_ps:
                for po in range(PO):
                    ps = pj_ps.tile([P, ROWS], f32, tag="pjps",
                                    name=f"qpq{po}")
                    for dint in range(PO):
                        nc.tensor.matmul(
                            ps[:],
                            wq_w[:, dint, po * P:(po + 1) * P],
                            xqT[:, dint, :],
                            start=(dint == 0), stop=(dint == PO - 1))
                    nc.vector.tensor_scalar_add(XqT[:, po, :], ps[:],
                                                bq_sb[:, po:po + 1])
                for po in range(PO):
                    ps = pj_ps.tile([P, ROWS], f32, tag="pjps",
                                    name=f"kpq{po}")
                    for dint in range(PO):
                        nc.tensor.matmul(
                            ps[:],
                            wk_w[:, dint, po * P:(po + 1) * P],
                            xkT[:, dint, :],
                            start=(dint == 0), stop=(dint == PO - 1))
                    nc.vector.tensor_scalar_add(XkT[:, po, :], ps[:],
                                                bk_sb[:, po:po + 1])
                    nc.scalar.dma_start(out=XkTs[0:64, po, :],
                                        in_=XkT[64:128, po, :])
                    nc.scalar.dma_start(out=XkTs[64:128, po, :],
                                        in_=XkT[0:64, po, :])

            # ------------- attention (V-proj drip-fed) -------------
            with tc.tile_pool(name="sc_ps", bufs=2, space="PSUM") as sc_ps, \
                 tc.tile_pool(name="av_ps", bufs=1, space="PSUM") as av_ps, \
                 tc.tile_pool(name="pv_tr", bufs=2, space="PSUM") as pv_tr, \
                 tc.tile_pool(name="attn", bufs=24) as attn_pool, \
                 tc.tile_pool(name="fin", bufs=4) as fin_pool:

                # Two deferred-work queues drained between bk groups:
                #   av_queue: AV matmuls (exactly 1 per bk -> AV lags scores
                #             by a few bk, so the PE never stalls on exp)
                #   bg_queue: V-proj chunks, finish chains, out-DMAs
                av_queue = []
                bg_queue = []

                def drain_av():
                    if av_queue:
                        av_queue.pop(0)()

                def drain_bg(n=1):
                    n += (len(bg_queue) > 12)
                    for _ in range(n):
                        if bg_queue:
                            bg_queue.pop(0)()

                # V projection: fine-grained items (2 matmuls each, ~210ns)
                # so a drain slot never displaces a score matmul by much.
                def v_proj_items(h):
                    items = []
                    state = {}
                    for dhalf in range(2):
                        for qh in range(2):
                            qd = dhalf * 2 + qh
                            for dp in range(PO // 2):
                                def mm(h=h, dhalf=dhalf, qh=qh, qd=qd, dp=dp):
                                    if dhalf not in state:
                                        state[dhalf] = pv_tr.tile(
                                            [P, ROWS], f32, tag="pvtr",
                                            name=f"vps{h}_{dhalf}")
                                    ps = state[dhalf]
                                    for dint in (2 * dp, 2 * dp + 1):
                                        nc.tensor.matmul(
                                            ps[:, qh * (ROWS // 2):(qh + 1) * (ROWS // 2)],
                                            xvT[:, dint, h * P:(h + 1) * P],
                                            wv_w[:, dint, qd * (D // 4):(qd + 1) * (D // 4)],
                                            start=(dint == 0),
                                            stop=(dint == PO - 1))
                                items.append(mm)

                        def evict(h=h, dhalf=dhalf):
                            ps = state.pop(dhalf)
                            nc.vector.tensor_add(
                                av_lhs[:, h, dhalf * 8:(dhalf + 1) * 8, 0:DH],
                                ps[:],
                                bv_sb[:, dhalf * ROWS:(dhalf + 1) * ROWS])
                        items.append(evict)
                    return items

                # K-rest projection drip items, per head block (hb=1 alone
                # first — its deadline is unit h1 — then h2+h3 together).
                def k_rest_items(m0, m1):
                    items = []
                    state = {}
                    mw = m1 - m0
                    for po in range(PO):
                        for dp in range(2):
                            def mm(po=po, dp=dp):
                                if po not in state:
                                    state[po] = pv_tr.tile(
                                        [P, ROWS], f32, tag="pvtr",
                                        name=f"krest{m0}_{po}")
                                ps = state[po]
                                for dint in range(4 * dp, 4 * dp + 4):
                                    nc.tensor.matmul(
                                        ps[:, 0:mw],
                                        wk_w[:, dint, po * P:(po + 1) * P],
                                        xkT[:, dint, m0:m1],
                                        start=(dint == 0),
                                        stop=(dint == PO - 1))
                            items.append(mm)

                        def evict_swap(po=po):
                            ps = state.pop(po)
                            nc.vector.tensor_scalar_add(
                                XkT[:, po, m0:m1], ps[:, 0:mw],
                                bk_sb[:, po:po + 1])
                            nc.gpsimd.dma_start(out=XkTs[0:64, po, m0:m1],
                                                in_=XkT[64:128, po, m0:m1])
                            nc.gpsimd.dma_start(out=XkTs[64:128, po, m0:m1],
                                                in_=XkT[0:64, po, m0:m1])
                        items.append(evict_swap)
                    return items

                # V(h0) fully before attention.
                for item in v_proj_items(0):
                    item()

                def unit(h, Hh, last=False):
                    out_ps = av_ps.tile([DH + 1, MH], f32, tag="avps")
                    bk_order = ([b_ for b_ in range(KB) if b_ % 2 == Hh] +
                                [b_ for b_ in range(KB) if b_ % 2 != Hh])
                    for bki, bkk in enumerate(bk_order):
                        ksrc = XkT if (bkk % 2) == Hh else XkTs
                        lhsT = ksrc[Hh * 64:Hh * 64 + 64, bkk // 2,
                                    h * P:(h + 1) * P]
                        s_ps = sc_ps.tile([P, MH], f32, tag="scps")
                        for j2 in range(2):
                            rhs = XqT[Hh * 64:Hh * 64 + 64,
                                      4 * j2:4 * j2 + 4, h * P:(h + 1) * P]
                            nc.tensor.matmul(
                                s_ps[:, j2 * ROWS:(j2 + 1) * ROWS],
                                lhsT, rhs, start=True, stop=True)
                        at = attn_pool.tile([P, MH], bf16, tag="at")
                        nc.scalar.activation(at[:], s_ps[:], Exp, scale=1.0)

                        def av(bki=bki, bkk=bkk, at=at, out_ps=out_ps, h=h):
                            for j2 in range(2):
                                nc.tensor.matmul(
                                    out_ps[:, j2 * ROWS:(j2 + 1) * ROWS],
                                    av_lhs[:, h, bkk, :],
                                    at[:, j2 * ROWS:(j2 + 1) * ROWS],
                                    start=(bki == 0), stop=(bki == KB - 1))
                        av_queue.append(av)
                        drain_av()
                        drain_bg(2 if last else 1)

                    # finish chain: deferred (drains during the next unit).
                    # copy_out has a hard deadline (av_ps bufs=1 WAR with the
                    # next unit's first AV) so unit() queues it itself; the
                    # caller queues fin chunks after any V-proj items.
                    def copy_out(out_ps=out_ps, h=h, Hh=Hh):
                        oT = fin_pool.tile([DH + 1, MH], bf16, tag="oT", bufs=2)
                        nc.vector.tensor_copy(oT[:], out_ps[:])
                        finish_state[(h, Hh)] = oT

                    def fin_chunk(j, h=h, Hh=Hh):
                        def run():
                            oT = finish_state[(h, Hh)]
                            stage = finish_state.get((h, Hh, "stage"))
                            if stage is None:
                                stage = fin_pool.tile([P, MH // P, DH], f32,
                                                      tag="stage", bufs=2,
                                                      name=f"stage{h}_{Hh}")
                                finish_state[(h, Hh, "stage")] = stage
                            tp = pv_tr.tile([P, DH + 1], bf16, tag="pvtr")
                            nc.tensor.transpose(tp[:], oT[:, j * P:(j + 1) * P],
                                                ident[0:DH + 1, 0:DH + 1])
                            rcp = fin_pool.tile([P, 1], f32, tag="rcp")
                            nc.vector.reciprocal(rcp[:], tp[:, DH:DH + 1])
                            nc.vector.tensor_scalar_mul(stage[:, j, :],
                                                        tp[:, 0:DH], rcp[:])
                        return run

                    def dma_out(h=h, Hh=Hh):
                        stage = finish_state.pop((h, Hh, "stage"))
                        finish_state.pop((h, Hh))
                        nc.sync.dma_start(out=out_w[h, :, Hh::2, :], in_=stage[:])

                    # queue-jump: copy_out must drain at the very next bk
                    # (av_ps bufs=1 WAR with the next unit's first AV)
                    bg_queue.insert(0, copy_out)
                    return [fin_chunk(j) for j in range(MH // P)] + [dma_out]

                finish_state = {}
                pending = []
                # drip schedule balanced so no unit gets more than ~32 items:
                #   (h0,0): K-h1   (h0,1): V-h1   (h1,0): K-h2h3
                #   (h1,1): V-h2   (h2,1): V-h3
                for h in range(HPC):
                    for Hh in range(2):
                        if Hh == 1 and h + 1 < HPC:
                            bg_queue.extend(v_proj_items(h + 1))
                        bg_queue.extend(pending)
                        pending = unit(h, Hh, last=(h == HPC - 1))
                # tail: drain everything that's left
                while av_queue:
                    drain_av()
                bg_queue.extend(pending)
                while bg_queue:
                    bg_queue.pop(0)()

            inputs_qk.__exit__(None, None, None)

    nc.compile()
    _dedupe_ldweights(nc)
    _BUILT[("nc", reps, salt)] = nc
    return nc


def _dedupe_ldweights(nc):
    """Remove InstLdweights that reload the stationary already resident in
    the PE array (consecutive matmuls sharing lhsT). Conservative: only
    sync-free duplicates; tracking resets at transposes (which clobber the
    array), drains, branches and any synced reload."""
    def key(a):
        return (str(a.memref), a.offset, str(a.ap), str(a.dtype))

    for f in nc.m.functions:
        for b in f.blocks:
            last = None
            keep = []
            for i in b.instructions:
                tn = type(i).__name__
                if tn == "InstLdweights":
                    k = key(i.ins[0])
                    si = i.sync_info
                    clean = (si is None) or (not si.on_wait and not si.on_update)
                    if last == k and clean:
                        continue
                    last = k
                elif tn == "InstMatmult":
                    if i.is_transpose:
                        last = None
                elif tn in ("InstDrain", "InstUnconditionalBranch", "InstCall"):
                    last = None
                keep.append(i)
            b.instructions[:] = keep


def _make_in_maps(q, k, v, Wq, bq, Wk, bk, Wv, bv):
    import ml_dtypes
    bfl = ml_dtypes.bfloat16

    q = np.asarray(q, dtype=np.float32)
    k = np.asarray(k, dtype=np.float32)
    v = np.asarray(v, dtype=np.float32)
    # fold the 1/sqrt(DH)=1/8 softmax scale into Wq/bq (exact: power of 2)
    Wq_b = np.ascontiguousarray((np.asarray(Wq, np.float32) * 0.125).astype(bfl))
    Wk_b = np.ascontiguousarray(np.asarray(Wk, np.float32).astype(bfl))
    Wv_b = np.ascontiguousarray(np.asarray(Wv, np.float32).astype(bfl))
    bq_t = np.ascontiguousarray(
        (np.asarray(bq, np.float32) * 0.125).reshape(PO, 128).T)
    bk_t = np.ascontiguousarray(np.asarray(bk, np.float32).reshape(PO, 128).T)
    bv_t = np.ascontiguousarray(np.asarray(bv, np.float32).reshape(1, D))

    in_maps = []
    for c in range(NCORES):
        b = c // (NCORES // B)
        r0 = (c % (NCORES // B)) * ROWS
        in_maps.append({
            "qT": np.ascontiguousarray(q[b, r0:r0 + ROWS, :].T.astype(bfl)),
            "kT": np.ascontiguousarray(k[b, r0:r0 + ROWS, :].T.astype(bfl)),
            "vT": np.ascontiguousarray(v[b, r0:r0 + ROWS, :].T.astype(bfl)),
            "Wq": Wq_b, "Wk": Wk_b, "Wv": Wv_b,
            "bq": bq_t, "bk": bk_t, "bv": bv_t,
        })
    return in_maps


def kernel(q, k, v, Wq, bq, Wk, bk, Wv, bv):
    from concourse.bass_utils import run_bass_kernel_spmd

    nc = _build_nc()
    in_maps = _make_in_maps(q, k, v, Wq, bq, Wk, bk, Wv, bv)
    res = run_bass_kernel_spmd(nc, in_maps, core_ids=list(range(NCORES)))

    outp = np.empty((B, NH, S, DH), dtype=np.float32)
    for c in range(NCORES):
        b = c // (NCORES // B)
        h0 = (c % (NCORES // B)) * HPC
        outp[b, h0:h0 + HPC] = res.results[c]["out"]
    return outp
